# revision 8
# baseline (speedup 1.0000x reference)
"""Trainium2 Bass kernel for nn_NodeAttnModel (GATv2Conv + norm + MLP).

Strategy (8 NeuronCores, no collectives):
  - Shard by destination node range: core k owns nodes [k*6250, (k+1)*6250)
    and every edge pointing at them; segment softmax / aggregation is local.
  - Each core builds the full xl = x@Wl table in HBM (bf16, 512B rows) and
    gathers xl[src] rows per edge with gpsimd dma_gather, rotated across the
    4 SWDGE queues so descriptor generation runs on all Q7 core pairs.
  - Edges are grouped by 112-node destination blocks; per 128-edge chunk the
    per-edge sum v = xl[src] + xr[dst] + ea is two matmuls:
       s = [mask; eaT]^T @ [xr_block; We]  +  I^T @ xl_gather
    leaky_relu runs on the scalar engine (Lrelu) straight out of PSUM, the
    attention dot on vector (mult+reduce), exp(score) is emitted pre-broadcast
    by the scalar engine, and aggregation is one matmul per chunk.
  - Node phase (residual+LN+MLP+LN+residual+LN) runs in groups of 4 tiles
    with batched DMAs and table-set-aware op ordering.
"""

import math

import numpy as np
import ml_dtypes

import concourse.bass as bass
import concourse.bacc as bacc
import concourse.mybir as mybir
import concourse.tile as tile
from concourse.bass_utils import run_bass_kernel_spmd

BF = ml_dtypes.bfloat16
F32 = mybir.dt.float32
BF16 = mybir.dt.bfloat16
I16 = mybir.dt.int16
U8 = mybir.dt.uint8
AL = mybir.AluOpType
AF = mybir.ActivationFunctionType

# Problem constants
N, D, H, C, E, ED, HID = 50000, 160, 5, 32, 800000, 16, 512
EPS = 1e-5
SLOPE = 0.2
SELU_L = 1.0507009873554805
SELU_A = 1.6732632423543772

NCORES = 8
P = 128
BLK = 112          # dst nodes per block (112 + 16 edge-feature rows = 128 = K)
CPT = 8            # chunks (of 128 edges) per tile
DEAD = 1000.0      # dst_rel sentinel for padding edges
EW = 256           # xl-table row width in bf16 (512 B, dma_gather elem_size)
G1 = 8             # xl-table tiles per build group
G2 = 8             # xr blocks per build group
NA = 4             # node tiles per group

# aux pack: [drow bf16 2048B (rows 0:112) / eaT bf16 (rows 112:128)
#            | gidx int16 128B | drelT f32 32B]
AUX_GIDX = CPT * P * 2                # 2048: drow/eaT region (bf16)
AUX_DREL = AUX_GIDX + (CPT * P // 16) * 2  # + 128: gidx region (int16)
AUXB = AUX_DREL + CPT * 4             # 2176 + 32 = 2208


class Cfg:
    def __init__(self, n=N, e=E, ncores=NCORES):
        self.N, self.E, self.NCORES = n, e, ncores
        self.NV = n // ncores                    # nodes per core
        self.NBLK = math.ceil(self.NV / BLK)     # blocks per core
        self.NPAD = self.NBLK * BLK              # padded nodes per core
        assert self.NPAD % P == 0, (self.NPAD, "node pad must be 128-divisible")
        self.NTILE = self.NPAD // P              # node-phase tiles
        self.NG = math.ceil(n / (P * G1))        # table build groups
        self.TROWS = self.NG * G1 * P            # xl table rows
        self.SPLIT = self.TROWS // 2             # lo/hi table split (int16 idx)
        assert self.SPLIT < 32768 and self.TROWS - self.SPLIT < 32768
        self.NBG = math.ceil(self.NBLK / G2)     # xr build groups


def _prep_edges(cfg, edge_index, edge_attr):
    """Sort/pad edges into the uniform per-core block/chunk structure.

    Within each (core, block), low-src edges (src < SPLIT) come first, then
    high-src edges; each group is padded to a multiple of 128 so every
    128-edge chunk gathers from a single half of the xl table."""
    src = np.asarray(edge_index[0]).astype(np.int64)
    dst = np.asarray(edge_index[1]).astype(np.int64)
    e = src.shape[0]
    core = dst // cfg.NV
    rel = dst - core * cfg.NV
    blk = rel // BLK
    lane = rel - blk * BLK
    half = (src >= cfg.SPLIT).astype(np.int64)
    gkey = (core * cfg.NBLK + blk) * 2 + half
    order = np.argsort(gkey, kind="stable")
    gcounts = np.bincount(gkey, minlength=cfg.NCORES * cfg.NBLK * 2)
    counts = gcounts.reshape(cfg.NCORES, cfg.NBLK, 2)
    chunks_per = -(-counts.max(axis=0) // P)          # [NBLK, 2]
    need = chunks_per.sum(axis=1) == 0
    chunks_per[need, 0] = 1
    S = int(chunks_per.sum())
    T = -(-S // CPT)
    S_pad = T * CPT
    chunk_blk = np.full(S_pad, cfg.NBLK - 1, np.int64)
    chunk_half = np.zeros(S_pad, np.int64)
    chunk_base = np.zeros((cfg.NBLK, 2), np.int64)
    pos = 0
    for b in range(cfg.NBLK):
        for h in range(2):
            chunk_base[b, h] = pos
            n = int(chunks_per[b, h])
            chunk_blk[pos:pos + n] = b
            chunk_half[pos:pos + n] = h
            pos += n
    first_chunk = np.zeros(cfg.NBLK, np.int64)
    last_chunk = np.zeros(cfg.NBLK, np.int64)
    for b in range(cfg.NBLK):
        w = np.nonzero(chunk_blk == b)[0]
        first_chunk[b], last_chunk[b] = w[0], w[-1]

    gstart = np.zeros_like(gcounts)
    gstart[1:] = np.cumsum(gcounts)[:-1]
    ranks = np.arange(e) - gstart[gkey[order]]
    ecore = core[order]
    epos = chunk_base[blk[order], half[order]] * P + ranks

    src_pad = np.zeros((cfg.NCORES, S_pad * P), np.int16)
    drel_pad = np.full((cfg.NCORES, S_pad * P), DEAD, np.float32)
    ea_pad = np.zeros((cfg.NCORES, S_pad * P, ED), np.float32)
    src_pad[ecore, epos] = (src[order] - half[order] * cfg.SPLIT).astype(np.int16)
    drel_pad[ecore, epos] = lane[order].astype(np.float32)
    ea_pad[ecore, epos] = np.asarray(edge_attr, np.float32)[order]

    # wrapped idx layout: flat slot k -> [k % 16, k // 16], replicated x8
    k = np.arange(CPT * P)
    wrapped = np.zeros((cfg.NCORES, T, 16, CPT * P // 16), np.int16)
    wrapped[:, :, k % 16, k // 16] = src_pad.reshape(cfg.NCORES, T, CPT * P)
    gidx = np.tile(wrapped, (1, 1, 8, 1))                 # [NC, T, 128, 64]

    drelT = drel_pad.reshape(cfg.NCORES, T, CPT, P).transpose(0, 1, 3, 2)
    drow = drel_pad.reshape(cfg.NCORES, T, CPT * P).astype(BF)
    eaT = ea_pad.reshape(cfg.NCORES, T, CPT * P, ED).transpose(0, 1, 3, 2)

    aux = np.zeros((cfg.NCORES, T, P, AUXB), np.uint8)
    aux[:, :, 0:BLK, 0:AUX_GIDX] = (
        drow[:, :, None, :].view(np.uint8).reshape(cfg.NCORES, T, 1, AUX_GIDX)
    )
    aux[:, :, BLK:P, 0:AUX_GIDX] = (
        np.ascontiguousarray(eaT.astype(BF)).view(np.uint8).reshape(
            cfg.NCORES, T, 16, AUX_GIDX)
    )
    aux[:, :, :, AUX_GIDX:AUX_DREL] = gidx.view(np.uint8).reshape(
        cfg.NCORES, T, P, 128)
    aux[:, :, :, AUX_DREL:AUXB] = (
        drelT.astype(np.float32).copy().view(np.uint8).reshape(
            cfg.NCORES, T, P, CPT * 4))

    # gather runs per tile: maximal same-half chunk ranges
    runs = []
    for t in range(T):
        rr = []
        a = 0
        for c in range(1, CPT + 1):
            if c == CPT or chunk_half[t * CPT + c] != chunk_half[t * CPT + a]:
                rr.append((int(chunk_half[t * CPT + a]), a, c))
                a = c
        runs.append(rr)

    sched = dict(
        T=T,
        chunk_blk=chunk_blk.tolist(),
        first_chunk=first_chunk.tolist(),
        last_chunk=last_chunk.tolist(),
        runs=runs,
    )
    return sched, np.ascontiguousarray(aux)


def _nontriv(a, v):
    return not np.all(np.asarray(a) == v)


def build_trace(cfg, sched, weights, phases=("table", "xr", "edge", "node")):
    """Build the Bass/Tile program (identical for all cores)."""
    T = sched["T"]
    chunk_blk = sched["chunk_blk"]
    last_chunk_of = {g: b for b, g in enumerate(sched["last_chunk"])}
    first_chunk_of = {g: b for b, g in enumerate(sched["first_chunk"])}

    W = weights
    use_bl = _nontriv(W["bl"], 0.0)
    use_br = _nontriv(W["br"], 0.0)
    use_bgat = _nontriv(W["b_gat"], 0.0)
    use_g1 = _nontriv(W["g1"], 1.0)
    use_b1 = _nontriv(W["b1"], 0.0)
    use_bm1 = _nontriv(W["b_m1"], 0.0)
    use_gm = _nontriv(W["g_m"], 1.0)
    use_bm = _nontriv(W["b_m"], 0.0)
    use_bm2 = _nontriv(W["b_m2"], 0.0)
    use_g2 = _nontriv(W["g2"], 1.0)
    use_b2 = _nontriv(W["b2"], 0.0)

    nc = bacc.Bacc("TRN2", target_bir_lowering=False, debug=False,
                   num_swdge_queues=4)

    # ---------------- I/O declarations ----------------
    d_aux = nc.dram_tensor("aux", [T, P, AUXB], U8, kind="ExternalInput")
    d_xtg_hi = nc.dram_tensor("xtg_hi", [cfg.NG, P, G1 * P], BF16,
                              kind="ExternalInput")
    d_xtg_lo = nc.dram_tensor("xtg_lo", [cfg.NG, D - P, G1 * P], BF16,
                              kind="ExternalInput")
    d_xog_hi = nc.dram_tensor("xog_hi", [cfg.NBG, P, G2 * BLK], BF16,
                              kind="ExternalInput")
    d_xog_lo = nc.dram_tensor("xog_lo", [cfg.NBG, D - P, G2 * BLK], BF16,
                              kind="ExternalInput")
    d_xown = nc.dram_tensor("x_own", [cfg.NPAD, D], F32, kind="ExternalInput")
    d_out = nc.dram_tensor("y_out", [cfg.NPAD, D], F32, kind="ExternalOutput")

    def inline(arr, name):
        return nc.inline_tensor(np.ascontiguousarray(arr), name=name)

    bf = lambda a: np.asarray(a, np.float32).astype(BF)
    c_Wl_hi = inline(bf(W["Wl"][0:P, :]), "c_Wl_hi")
    c_Wl_lo = inline(bf(W["Wl"][P:D, :]), "c_Wl_lo")
    c_Wr_hi = inline(bf(W["Wr"][0:P, :]), "c_Wr_hi")
    c_Wr_lo = inline(bf(W["Wr"][P:D, :]), "c_Wr_lo")
    c_We = inline(bf(W["We"]), "c_We")
    c_ident = inline(np.eye(P, dtype=BF), "c_ident")
    c_att = inline(np.broadcast_to(
        bf(np.asarray(W["att"]).reshape(1, D)), (P, D)).copy(), "c_att")
    c_iota_c = inline(np.arange(BLK, dtype=np.float32).reshape(BLK, 1),
                      "c_iota_c")
    c_iota_r = inline(np.broadcast_to(
        np.arange(P, dtype=np.float32).reshape(1, P).astype(BF),
        (P, P)).copy(), "c_iota_r")
    c_Wm1_hi = inline(bf(W["W_m1"][0:P, :]), "c_Wm1_hi")
    c_Wm1_lo = inline(bf(W["W_m1"][P:D, :]), "c_Wm1_lo")
    c_Wm2 = inline(
        bf(W["W_m2"]).reshape(4, P, D).transpose(1, 0, 2).copy(), "c_Wm2"
    )  # [128, 4, 160]
    rows32 = lambda a: np.broadcast_to(
        np.asarray(a, np.float32).reshape(1, -1), (P, np.asarray(a).size)
    ).copy()
    c_bl = inline(rows32(W["bl"]), "c_bl")
    c_br = inline(rows32(W["br"]), "c_br")
    c_bgat = inline(rows32(W["b_gat"]), "c_bgat")
    c_g1 = inline(rows32(W["g1"]), "c_g1")
    c_b1 = inline(rows32(W["b1"]), "c_b1")
    c_bm1 = inline(rows32(W["b_m1"]), "c_bm1")
    c_gm = inline(rows32(W["g_m"]), "c_gm")
    c_bm = inline(rows32(W["b_m"]), "c_bm")
    c_bm2 = inline(rows32(W["b_m2"]), "c_bm2")
    c_g2 = inline(rows32(W["g2"]), "c_g2")
    c_b2 = inline(rows32(W["b2"]), "c_b2")

    with tile.TileContext(nc) as tc:
        psp = tc.alloc_tile_pool(name="psp", bufs=8, space="PSUM")
        dram = tc.alloc_tile_pool(name="dram", bufs=1, space="DRAM")
        xl_table = dram.tile([cfg.TROWS, EW], BF16, name="xl_table",
                             tag="xl_table")
        agg_d = dram.tile([cfg.NPAD, D], F32, name="agg_d", tag="agg_d")

        cp = tc.alloc_tile_pool(name="consts", bufs=1)

        def csb(dr, shape, dtype, name):
            t = cp.tile(shape, dtype, name=name, tag=name)
            nc.sync.dma_start(out=t[tuple(slice(0, s) for s in shape)],
                              in_=dr[:])
            return t

        Wl_hi = csb(c_Wl_hi, [P, D], BF16, "Wl_hi")
        Wl_lo = csb(c_Wl_lo, [D - P, D], BF16, "Wl_lo")
        Wr_hi = csb(c_Wr_hi, [P, D], BF16, "Wr_hi")
        Wr_lo = csb(c_Wr_lo, [D - P, D], BF16, "Wr_lo")
        ident = csb(c_ident, [P, P], BF16, "ident")
        att_sb = csb(c_att, [P, D], BF16, "att_sb")
        iota_c = csb(c_iota_c, [BLK, 1], F32, "iota_c")
        iota_r = csb(c_iota_r, [P, P], BF16, "iota_r")
        Wm1_hi = csb(c_Wm1_hi, [P, HID], BF16, "Wm1_hi")
        Wm1_lo = csb(c_Wm1_lo, [D - P, HID], BF16, "Wm1_lo")
        Wm2_sb = csb(c_Wm2, [P, 4, D], BF16, "Wm2_sb")
        bl_sb = csb(c_bl, [P, D], F32, "bl_sb") if use_bl else None
        br_sb = csb(c_br, [P, D], F32, "br_sb") if use_br else None
        bgat_sb = csb(c_bgat, [P, D], F32, "bgat_sb") if use_bgat else None
        g1_sb = csb(c_g1, [P, D], F32, "g1_sb") if use_g1 else None
        b1_sb = csb(c_b1, [P, D], F32, "b1_sb") if use_b1 else None
        bm1_sb = csb(c_bm1, [P, HID], F32, "bm1_sb") if use_bm1 else None
        gm_sb = csb(c_gm, [P, HID], F32, "gm_sb") if use_gm else None
        bm_sb = csb(c_bm, [P, HID], F32, "bm_sb") if use_bm else None
        bm2_sb = csb(c_bm2, [P, D], F32, "bm2_sb") if use_bm2 else None
        g2_sb = csb(c_g2, [P, D], F32, "g2_sb") if use_g2 else None
        b2_sb = csb(c_b2, [P, D], F32, "b2_sb") if use_b2 else None
        eps_sb = cp.tile([P, 1], F32, name="eps_sb", tag="eps_sb")
        nc.gpsimd.memset(eps_sb[:, :], float(EPS))
        lna_sb = cp.tile([P, 1], F32, name="lna_sb", tag="lna_sb")
        nc.gpsimd.memset(lna_sb[:, :], float(math.log(SELU_L * SELU_A)))

        # xr per block + We rows, concatenated along free dim
        rhs_all = cp.tile([P, cfg.NBLK * D], BF16, name="rhs_all",
                          tag="rhs_all")
        nc.sync.dma_start(
            out=rhs_all[BLK:P, :].rearrange("p (b f) -> p b f", f=D),
            in_=c_We[:].rearrange("p f -> p () f").to_broadcast(
                [ED, cfg.NBLK, D]))

        # ---------------- phase 1a: xl table ----------------
        tp = tc.alloc_tile_pool(name="tbl", bufs=3)
        if "table" in phases:
            for g in range(cfg.NG):
                xt_hi = tp.tile([P, G1 * P], BF16, name=f"xt_hi{g}",
                                tag="xt_hi", bufs=3)
                xt_lo = tp.tile([D - P, G1 * P], BF16, name=f"xt_lo{g}",
                                tag="xt_lo", bufs=3)
                nc.sync.dma_start(out=xt_hi[:, :], in_=d_xtg_hi[g])
                nc.scalar.dma_start(out=xt_lo[:, :], in_=d_xtg_lo[g])
                xlb = tp.tile([P, G1, EW], BF16, name=f"xlb{g}", tag="xlb",
                              bufs=3)
                for a in range(G1):
                    ps = psp.tile([P, D], F32, name=f"ps_xl{g}_{a}", tag="ps")
                    nc.tensor.matmul(ps[:, :], xt_hi[:, a * P:(a + 1) * P],
                                     Wl_hi[:, :], start=True, stop=False)
                    nc.tensor.matmul(ps[:, :], xt_lo[:, a * P:(a + 1) * P],
                                     Wl_lo[:, :], start=False, stop=True)
                    dst = xlb[:, a, 0:D]
                    if use_bl:
                        nc.vector.tensor_tensor(out=dst, in0=ps[:, :],
                                                in1=bl_sb[:, :], op=AL.add)
                    elif a % 2 == 0:
                        nc.scalar.copy(out=dst, in_=ps[:, :])
                    else:
                        nc.vector.tensor_copy(out=dst, in_=ps[:, :])
                nc.sync.dma_start(
                    out=xl_table[g * G1 * P:(g + 1) * G1 * P, :].rearrange(
                        "(a p) w -> p a w", p=P),
                    in_=xlb[:, :, :])

        # ---------------- phase 1b: xr per block -> rhs_all ----------------
        xp = tc.alloc_tile_pool(name="xrp", bufs=3)
        if "xr" in phases:
            for bg in range(cfg.NBG):
                nb = min(G2, cfg.NBLK - bg * G2)
                xo_hi = xp.tile([P, G2 * BLK], BF16, name=f"xo_hi{bg}",
                                tag="xo_hi", bufs=2)
                xo_lo = xp.tile([D - P, G2 * BLK], BF16, name=f"xo_lo{bg}",
                                tag="xo_lo", bufs=2)
                nc.scalar.dma_start(out=xo_hi[:, :], in_=d_xog_hi[bg])
                nc.scalar.dma_start(out=xo_lo[:, :], in_=d_xog_lo[bg])
                for bb in range(nb):
                    b = bg * G2 + bb
                    ps = psp.tile([BLK, D], F32, name=f"ps_xr{b}", tag="ps")
                    nc.tensor.matmul(ps[:, :],
                                     xo_hi[:, bb * BLK:(bb + 1) * BLK],
                                     Wr_hi[:, :], start=True, stop=False)
                    nc.tensor.matmul(ps[:, :],
                                     xo_lo[:, bb * BLK:(bb + 1) * BLK],
                                     Wr_lo[:, :], start=False, stop=True)
                    dst = rhs_all[0:BLK, b * D:(b + 1) * D]
                    if use_br:
                        nc.vector.tensor_tensor(out=dst, in0=ps[:, :],
                                                in1=br_sb[0:BLK, :], op=AL.add)
                    elif bb % 2 == 0:
                        nc.scalar.copy(out=dst, in_=ps[:, :])
                    else:
                        nc.vector.tensor_copy(out=dst, in_=ps[:, :])

        tc.strict_bb_all_engine_barrier()

        # ---------------- phase 2: edges ----------------
        ep = tc.alloc_tile_pool(name="ep", bufs=1)
        if "edge" in phases:
            agg_tiles = {}
            for t in range(T):
                aux = ep.tile([P, AUXB], U8, name=f"aux{t}", tag="aux", bufs=4)
                nc.sync.dma_start(out=aux[:, :], in_=d_aux[t])
                maskea = aux[:, 0:AUX_GIDX].bitcast(BF16)      # [128, 1024]
                gidx_v = aux[:, AUX_GIDX:AUX_DREL].bitcast(I16)  # [128, 64]
                drel_v = aux[:, AUX_DREL:AUXB].bitcast(F32)      # [128, 8]

                # one-hot mask rows (in place over drow) + per-chunk m2
                nc.vector.tensor_scalar(
                    out=maskea[0:BLK, :], in0=maskea[0:BLK, :],
                    scalar1=iota_c[:, 0:1], scalar2=None, op0=AL.is_equal)
                m2 = ep.tile([P, CPT, P], BF16, name=f"m2_{t}", tag="m2",
                             bufs=3)
                for c in range(CPT):
                    nc.vector.tensor_scalar(
                        out=m2[:, c, :], in0=iota_r[:, :],
                        scalar1=drel_v[:, c:c + 1], scalar2=None,
                        op0=AL.is_equal)

                xlg = ep.tile([P, CPT, EW], BF16, name=f"xlg{t}", tag="xlg",
                              bufs=5)
                for (hf, a, b) in sched["runs"][t]:
                    nidx = P * (b - a)
                    nc.gpsimd.dma_gather(
                        out_ap=xlg[:, a:b, :],
                        in_ap=(xl_table[0:cfg.SPLIT, :] if hf == 0
                               else xl_table[cfg.SPLIT:cfg.TROWS, :]),
                        idxs_ap=gidx_v[:, a * CPT:b * CPT],
                        num_idxs=nidx, num_idxs_reg=nidx, elem_size=EW,
                        queue_num=t % 4)

                f_sb = ep.tile([P, CPT, D], BF16, name=f"f{t}", tag="f",
                               bufs=2)
                for c in range(CPT):
                    b = chunk_blk[t * CPT + c]
                    ps_s = psp.tile([P, D], F32, name=f"ps_s{t}_{c}", tag="ps")
                    nc.tensor.matmul(ps_s[:, :], maskea[:, c * P:(c + 1) * P],
                                     rhs_all[:, b * D:(b + 1) * D],
                                     start=True, stop=False)
                    nc.tensor.matmul(ps_s[:, :], ident[:, :], xlg[:, c, 0:D],
                                     start=False, stop=True)
                    nc.scalar.activation(out=f_sb[:, c, :], in_=ps_s[:, :],
                                         func=AF.Lrelu, alpha=float(SLOPE))

                # fa = f * att (in place), per-head reduce -> scores
                nc.vector.tensor_tensor(
                    out=f_sb[:, :, :], in0=f_sb[:, :, :],
                    in1=att_sb[:, :].rearrange("p f -> p () f").to_broadcast(
                        [P, CPT, D]),
                    op=AL.mult)
                sc = ep.tile([P, CPT, H], F32, name=f"sc{t}", tag="sc", bufs=2)
                nc.vector.tensor_reduce(
                    out=sc[:, :, :],
                    in_=f_sb[:, :, :].rearrange("p c (h z) -> p c h z", z=C),
                    axis=mybir.AxisListType.X, op=AL.add)

                w_sb = ep.tile([P, CPT, D + H], BF16, name=f"w{t}", tag="w",
                               bufs=2)
                nc.scalar.activation(
                    out=w_sb[:, :, 0:D].rearrange("p c (h z) -> p c h z", z=C),
                    in_=sc[:, :, :].rearrange("p c h -> p c h ()").to_broadcast(
                        [P, CPT, H, C]),
                    func=AF.Exp)
                nc.scalar.activation(out=w_sb[:, :, D:D + H], in_=sc[:, :, :],
                                     func=AF.Exp)
                nc.vector.tensor_tensor(
                    out=w_sb[:, :, 0:D], in0=w_sb[:, :, 0:D],
                    in1=xlg[:, :, 0:D], op=AL.mult)

                for c in range(CPT):
                    g = t * CPT + c
                    b = chunk_blk[g]
                    if g in first_chunk_of:
                        agg_tiles[b] = psp.tile([P, D + H], F32,
                                                name=f"agg{b}", tag="ps")
                    at = agg_tiles[b]
                    nc.tensor.matmul(at[:, :], m2[:, c, :], w_sb[:, c, :],
                                     start=(g in first_chunk_of),
                                     stop=(g in last_chunk_of))
                    if g in last_chunk_of:
                        den = ep.tile([BLK, H], F32, name=f"den{b}", tag="den",
                                      bufs=2)
                        nc.vector.tensor_scalar_add(
                            out=den[:, :], in0=at[0:BLK, D:D + H],
                            scalar1=1e-30)
                        rec = ep.tile([BLK, H], F32, name=f"rec{b}", tag="rec",
                                      bufs=2)
                        nc.vector.reciprocal(out=rec[:, :], in_=den[:, :])
                        aggn = ep.tile([BLK, D], F32, name=f"aggn{b}",
                                       tag="aggn", bufs=2)
                        nc.vector.tensor_tensor(
                            out=aggn[:, :].rearrange("p (h z) -> p h z", z=C),
                            in0=at[0:BLK, 0:D].rearrange(
                                "p (h z) -> p h z", z=C),
                            in1=rec[:, :].rearrange(
                                "p h -> p h ()").to_broadcast([BLK, H, C]),
                            op=AL.mult)
                        nc.sync.dma_start(
                            out=agg_d[b * BLK:(b + 1) * BLK, :],
                            in_=aggn[:, :])
                        del agg_tiles[b]

        tc.strict_bb_all_engine_barrier()

        # ---------------- phase 3: node MLP ----------------
        npo = tc.alloc_tile_pool(name="np", bufs=1)
        if "node" in phases:
            groups = [(j0, min(j0 + NA, cfg.NTILE))
                      for j0 in range(0, cfg.NTILE, NA)]
            for (j0, j1) in groups:
                A = j1 - j0
                rs = slice(j0 * P, j1 * P)
                x_t = npo.tile([P, NA, D], F32, name=f"x{j0}", tag="x", bufs=2)
                a_t = npo.tile([P, NA, D], F32, name=f"a{j0}", tag="a", bufs=2)
                nc.sync.dma_start(
                    out=x_t[:, 0:A, :],
                    in_=d_xown[rs, :].rearrange("(a p) d -> p a d", p=P))
                nc.sync.dma_start(
                    out=a_t[:, 0:A, :],
                    in_=agg_d[rs, :].rearrange("(a p) d -> p a d", p=P))
                t0 = npo.tile([P, NA, D], F32, name=f"t0_{j0}", tag="t0",
                              bufs=2)
                nc.vector.tensor_tensor(out=t0[:, 0:A, :], in0=x_t[:, 0:A, :],
                                        in1=a_t[:, 0:A, :], op=AL.add)
                if use_bgat:
                    for a in range(A):
                        nc.gpsimd.tensor_tensor(
                            out=t0[:, a, :], in0=t0[:, a, :],
                            in1=bgat_sb[:, :], op=AL.add)

                def ln_stats(src, width, A, nm):
                    """Per-slot mean/rstd for src [P, A, width]."""
                    st = npo.tile([P, NA, 6], F32, name=f"st{nm}",
                                  tag=f"st{nm[0]}", bufs=2)
                    mv = npo.tile([P, NA, 2], F32, name=f"mv{nm}",
                                  tag=f"mv{nm[0]}", bufs=2)
                    for a in range(A):
                        nc.vector.bn_stats(out=st[:, a, :], in_=src[:, a, :])
                        nc.vector.bn_aggr(out=mv[:, a, :], in_=st[:, a, :])
                    sd = npo.tile([P, NA], F32, name=f"sd{nm}",
                                  tag=f"sd{nm[0]}", bufs=2)
                    nc.scalar.activation(out=sd[:, 0:A], in_=mv[:, 0:A, 1],
                                         func=AF.Sqrt, bias=eps_sb[:, 0:1])
                    rstd = npo.tile([P, NA], F32, name=f"rstd{nm}",
                                    tag=f"rstd{nm[0]}", bufs=2)
                    nc.vector.reciprocal(out=rstd[:, 0:A], in_=sd[:, 0:A])
                    return mv, rstd

                # ---- LN1 (sqrt table) ----
                mv1, rstd1 = ln_stats(t0, D, A, f"1_{j0}")
                out1 = npo.tile([P, NA, D], F32, name=f"o1_{j0}", tag="o1",
                                bufs=2)
                for a in range(A):
                    nc.vector.scalar_tensor_tensor(
                        out=out1[:, a, :], in0=t0[:, a, :],
                        scalar=mv1[:, a, 0:1],
                        in1=rstd1[:, a:a + 1].to_broadcast([P, D]),
                        op0=AL.subtract, op1=AL.mult)
                    if use_g1:
                        nc.vector.tensor_tensor(out=out1[:, a, :],
                                                in0=out1[:, a, :],
                                                in1=g1_sb[:, :], op=AL.mult)
                    if use_b1:
                        nc.gpsimd.tensor_tensor(out=out1[:, a, :],
                                                in0=out1[:, a, :],
                                                in1=b1_sb[:, :], op=AL.add)
                o1b = npo.tile([P, NA, D], BF16, name=f"o1b{j0}", tag="o1b",
                               bufs=2)
                nc.scalar.copy(out=o1b[:, 0:A, :], in_=out1[:, 0:A, :])

                # ---- transposes + mm1 + selu (exp table) ----
                e_g = npo.tile([P, NA, HID], BF16, name=f"e{j0}", tag="e",
                               bufs=2)
                r_g = npo.tile([P, NA, HID], BF16, name=f"r{j0}", tag="r",
                               bufs=2)
                for a in range(A):
                    pt0 = psp.tile([P, P], BF16, name=f"pt0_{j0}_{a}",
                                   tag="ps")
                    nc.tensor.transpose(out=pt0[:, :], in_=o1b[:, a, 0:P],
                                        identity=ident[:, :])
                    t0s = npo.tile([P, P], BF16, name=f"t0s{j0}_{a}",
                                   tag="t0s", bufs=3)
                    nc.scalar.copy(out=t0s[:, :], in_=pt0[:, :])
                    pt1 = psp.tile([D - P, P], BF16, name=f"pt1_{j0}_{a}",
                                   tag="ps")
                    nc.tensor.transpose(out=pt1[:, :], in_=o1b[:, a, P:D],
                                        identity=ident[:, :])
                    t1s = npo.tile([D - P, P], BF16, name=f"t1s{j0}_{a}",
                                   tag="t1s", bufs=3)
                    nc.vector.tensor_copy(out=t1s[:, :], in_=pt1[:, :])
                    ps_h = psp.tile([P, HID], F32, name=f"ps_h{j0}_{a}",
                                    tag="ps")
                    nc.tensor.matmul(ps_h[:, :], t0s[:, :], Wm1_hi[:, :],
                                     start=True, stop=False)
                    nc.tensor.matmul(ps_h[:, :], t1s[:, :], Wm1_lo[:, :],
                                     start=False, stop=True)
                    src = ps_h[:, :]
                    if use_bm1:
                        y_sb = npo.tile([P, HID], F32, name=f"ysb{j0}_{a}",
                                        tag="ysb", bufs=2)
                        nc.vector.tensor_tensor(out=y_sb[:, :], in0=src,
                                                in1=bm1_sb[:, :], op=AL.add)
                        src = y_sb[:, :]
                    nc.scalar.activation(out=e_g[:, a, :], in_=src,
                                         func=AF.Exp, bias=lna_sb[:, 0:1])
                    nc.scalar.activation(out=r_g[:, a, :], in_=src,
                                         func=AF.Relu, scale=float(SELU_L))
                u2 = npo.tile([P, NA, HID], BF16, name=f"u2_{j0}", tag="u2",
                              bufs=2)
                nc.vector.scalar_tensor_tensor(
                    out=u2[:, 0:A, :], in0=e_g[:, 0:A, :],
                    scalar=float(SELU_L * SELU_A),
                    in1=r_g[:, 0:A, :], op0=AL.min, op1=AL.add)

                # ---- LN2 (sqrt table + identity-scale) ----
                mv2, rstd2 = ln_stats(u2, HID, A, f"2_{j0}")
                nb2 = npo.tile([P, NA], F32, name=f"nb2_{j0}", tag="nb2",
                               bufs=2)
                nc.vector.scalar_tensor_tensor(
                    out=nb2[:, 0:A], in0=mv2[:, 0:A, 0], scalar=-1.0,
                    in1=rstd2[:, 0:A], op0=AL.mult, op1=AL.mult)
                h_bf = npo.tile([P, NA, HID], BF16, name=f"h{j0}", tag="h",
                                bufs=2)
                for a in range(A):
                    nc.scalar.activation(
                        out=h_bf[:, a, :], in_=u2[:, a, :], func=AF.Identity,
                        bias=nb2[:, a:a + 1], scale=rstd2[:, a:a + 1])
                    if use_gm:
                        nc.vector.tensor_tensor(out=h_bf[:, a, :],
                                                in0=h_bf[:, a, :],
                                                in1=gm_sb[:, :], op=AL.mult)
                    if use_bm:
                        nc.gpsimd.tensor_tensor(out=h_bf[:, a, :],
                                                in0=h_bf[:, a, :],
                                                in1=bm_sb[:, :], op=AL.add)

                # ---- transposes + mm2 + residual ----
                t2 = npo.tile([P, NA, D], F32, name=f"t2_{j0}", tag="t2",
                              bufs=2)
                for a in range(A):
                    ps_m = psp.tile([P, D], F32, name=f"ps_m{j0}_{a}",
                                    tag="ps")
                    for k in range(4):
                        pth = psp.tile([P, P], BF16, name=f"pth{j0}_{a}_{k}",
                                       tag="ps")
                        nc.tensor.transpose(out=pth[:, :],
                                            in_=h_bf[:, a, k * P:(k + 1) * P],
                                            identity=ident[:, :])
                        hts = npo.tile([P, P], BF16, name=f"hts{j0}_{a}_{k}",
                                       tag="hts", bufs=4)
                        if k % 2 == 0:
                            nc.scalar.copy(out=hts[:, :], in_=pth[:, :])
                        else:
                            nc.vector.tensor_copy(out=hts[:, :], in_=pth[:, :])
                        nc.tensor.matmul(ps_m[:, :], hts[:, :],
                                         Wm2_sb[:, k, :],
                                         start=(k == 0), stop=(k == 3))
                    nc.vector.tensor_tensor(out=t2[:, a, :],
                                            in0=out1[:, a, :],
                                            in1=ps_m[:, :], op=AL.add)
                    if use_bm2:
                        nc.gpsimd.tensor_tensor(out=t2[:, a, :],
                                                in0=t2[:, a, :],
                                                in1=bm2_sb[:, :], op=AL.add)

                # ---- LN3 (sqrt table) ----
                mv3, rstd3 = ln_stats(t2, D, A, f"3_{j0}")
                y_g = npo.tile([P, NA, D], F32, name=f"y{j0}", tag="y",
                               bufs=2)
                for a in range(A):
                    nc.vector.scalar_tensor_tensor(
                        out=y_g[:, a, :], in0=t2[:, a, :],
                        scalar=mv3[:, a, 0:1],
                        in1=rstd3[:, a:a + 1].to_broadcast([P, D]),
                        op0=AL.subtract, op1=AL.mult)
                    if use_g2:
                        nc.vector.tensor_tensor(out=y_g[:, a, :],
                                                in0=y_g[:, a, :],
                                                in1=g2_sb[:, :], op=AL.mult)
                    if use_b2:
                        nc.gpsimd.tensor_tensor(out=y_g[:, a, :],
                                                in0=y_g[:, a, :],
                                                in1=b2_sb[:, :], op=AL.add)
                nc.sync.dma_start(
                    out=d_out[rs, :].rearrange("(a p) d -> p a d", p=P),
                    in_=y_g[:, 0:A, :])

        npo.release()
        ep.release()
        xp.release()
        tp.release()
        psp.release()
        dram.release()
        cp.release()

    nc.compile()
    return nc


def _make_in_maps(cfg, x, aux):
    x32 = np.asarray(x, np.float32)
    xbf = x32.astype(BF)
    xT = np.zeros((D, cfg.TROWS), BF)
    xT[:, :cfg.N] = xbf.T
    # fused tile groups: [NG, feat, G1, 128] -> [NG, feat, G1*128]
    xtg = xT.reshape(D, cfg.NG, G1 * P).transpose(1, 0, 2)
    in_maps = []
    for k in range(cfg.NCORES):
        xo = np.zeros((cfg.NPAD, D), np.float32)
        xo[:cfg.NV] = x32[k * cfg.NV:(k + 1) * cfg.NV]
        xoT = np.zeros((D, cfg.NBG * G2 * BLK), BF)
        xoT[:, :cfg.NV] = xbf[k * cfg.NV:(k + 1) * cfg.NV].T
        xog = xoT.reshape(D, cfg.NBG, G2 * BLK).transpose(1, 0, 2)
        in_maps.append({
            "aux": aux[k],
            "xtg_hi": np.ascontiguousarray(xtg[:, 0:P]),
            "xtg_lo": np.ascontiguousarray(xtg[:, P:D]),
            "xog_hi": np.ascontiguousarray(xog[:, 0:P]),
            "xog_lo": np.ascontiguousarray(xog[:, P:D]),
            "x_own": xo,
        })
    return in_maps


def build_all(inputs, cfg=None, phases=("table", "xr", "edge", "node")):
    cfg = cfg or Cfg()
    sched, aux = _prep_edges(cfg, inputs["edge_index"], inputs["edge_attr"])
    wnames = ["Wl", "bl", "Wr", "br", "We", "att", "b_gat", "g1", "b1",
              "W_m1", "b_m1", "g_m", "b_m", "W_m2", "b_m2", "g2", "b2"]
    weights = {k: np.asarray(inputs[k], np.float32) for k in wnames}
    nc = build_trace(cfg, sched, weights, phases=phases)
    in_maps = _make_in_maps(cfg, inputs["x"], aux)
    return cfg, nc, in_maps


def kernel(**inputs) -> np.ndarray:
    cfg, nc, in_maps = build_all(inputs)
    res = run_bass_kernel_spmd(nc, in_maps, core_ids=list(range(cfg.NCORES)))
    out = np.concatenate(
        [r["y_out"][:cfg.NV] for r in res.results], axis=0
    ).astype(np.float32)
    return out


# revision 14
# speedup vs baseline: 1.1257x; 1.1257x over previous
"""Trainium2 Bass kernel for nn_NodeAttnModel (GATv2Conv + norm + MLP).

Strategy (8 NeuronCores, no collectives):
  - Shard by destination node range: core k owns nodes [k*6250, (k+1)*6250)
    and every edge pointing at them; segment softmax / aggregation is local.
  - Each core builds the full xl = x@Wl table in HBM (bf16, 512B rows) and
    gathers xl[src] rows per edge with gpsimd dma_gather, rotated across the
    4 SWDGE queues so descriptor generation runs on all Q7 core pairs.
  - Edges are grouped by 112-node destination blocks; per 128-edge chunk the
    per-edge sum v = xl[src] + xr[dst] + ea is two matmuls:
       s = [mask; eaT]^T @ [xr_block; We]  +  I^T @ xl_gather
    leaky_relu runs on the scalar engine (Lrelu) straight out of PSUM, the
    attention dot on vector (mult+reduce), exp(score) is emitted pre-broadcast
    by the scalar engine, and aggregation is one matmul per chunk.
  - Node phase (residual+LN+MLP+LN+residual+LN) runs in groups of 4 tiles
    with batched DMAs and table-set-aware op ordering.
"""

import math

import numpy as np
import ml_dtypes

import concourse.bass as bass
import concourse.bacc as bacc
import concourse.mybir as mybir
import concourse.tile as tile
from concourse.bass_utils import run_bass_kernel_spmd

BF = ml_dtypes.bfloat16
F32 = mybir.dt.float32
BF16 = mybir.dt.bfloat16
I16 = mybir.dt.int16
U8 = mybir.dt.uint8
AL = mybir.AluOpType
AF = mybir.ActivationFunctionType

# Problem constants
N, D, H, C, E, ED, HID = 50000, 160, 5, 32, 800000, 16, 512
EPS = 1e-5
SLOPE = 0.2
SELU_L = 1.0507009873554805
SELU_A = 1.6732632423543772

NCORES = 8
P = 128
BLK = 112          # dst nodes per block (112 + 16 edge-feature rows = 128 = K)
CPT = 8            # chunks (of 128 edges) per tile
DEAD = 1000.0      # dst_rel sentinel for padding edges
EW = 256           # xl-table row width in bf16 (512 B, dma_gather elem_size)
G1 = 8             # xl-table tiles per build group
G2 = 8             # xr blocks per build group
NA = 4             # node tiles per group

# aux pack: [drow bf16 2048B (rows 0:112) / eaT bf16 (rows 112:128)
#            | gidx int16 128B | drelT f32 32B]
AUX_GIDX = CPT * P * 2                # 2048: drow/eaT region (bf16)
AUX_DREL = AUX_GIDX + (CPT * P // 16) * 2  # + 128: gidx region (int16)
AUXB = AUX_DREL + CPT * 2             # 2176 + 16 = 2192 (drelT bf16)


class Cfg:
    def __init__(self, n=N, e=E, ncores=NCORES):
        self.N, self.E, self.NCORES = n, e, ncores
        self.NV = n // ncores                    # nodes per core
        self.NBLK = math.ceil(self.NV / BLK)     # blocks per core
        self.NPAD = self.NBLK * BLK              # padded nodes per core
        assert self.NPAD % P == 0, (self.NPAD, "node pad must be 128-divisible")
        self.NTILE = self.NPAD // P              # node-phase tiles
        self.NG = math.ceil(n / (P * G1))        # table build groups
        self.TROWS = self.NG * G1 * P            # xl table rows
        self.SPLIT = self.TROWS // 2             # lo/hi table split (int16 idx)
        assert self.SPLIT < 32768 and self.TROWS - self.SPLIT < 32768
        self.NBG = math.ceil(self.NBLK / G2)     # xr build groups


def _prep_edges(cfg, edge_index, edge_attr):
    """Sort/pad edges into the uniform per-core block/chunk structure.

    Within each (core, block), low-src edges (src < SPLIT) come first, then
    high-src edges; each group is padded to a multiple of 128 so every
    128-edge chunk gathers from a single half of the xl table."""
    src = np.asarray(edge_index[0]).astype(np.int64)
    dst = np.asarray(edge_index[1]).astype(np.int64)
    e = src.shape[0]
    core = dst // cfg.NV
    rel = dst - core * cfg.NV
    blk = rel // BLK
    lane = rel - blk * BLK
    half = (src >= cfg.SPLIT).astype(np.int64)
    gkey = (core * cfg.NBLK + blk) * 2 + half
    order = np.argsort(gkey, kind="stable")
    gcounts = np.bincount(gkey, minlength=cfg.NCORES * cfg.NBLK * 2)
    counts = gcounts.reshape(cfg.NCORES, cfg.NBLK, 2)
    chunks_per = -(-counts.max(axis=0) // P)          # [NBLK, 2]
    need = chunks_per.sum(axis=1) == 0
    chunks_per[need, 0] = 1
    S = int(chunks_per.sum())
    T = -(-S // CPT)
    S_pad = T * CPT
    chunk_blk = np.full(S_pad, cfg.NBLK - 1, np.int64)
    chunk_half = np.zeros(S_pad, np.int64)
    chunk_base = np.zeros((cfg.NBLK, 2), np.int64)
    pos = 0
    for b in range(cfg.NBLK):
        for h in range(2):
            chunk_base[b, h] = pos
            n = int(chunks_per[b, h])
            chunk_blk[pos:pos + n] = b
            chunk_half[pos:pos + n] = h
            pos += n
    first_chunk = np.zeros(cfg.NBLK, np.int64)
    last_chunk = np.zeros(cfg.NBLK, np.int64)
    for b in range(cfg.NBLK):
        w = np.nonzero(chunk_blk == b)[0]
        first_chunk[b], last_chunk[b] = w[0], w[-1]

    gstart = np.zeros_like(gcounts)
    gstart[1:] = np.cumsum(gcounts)[:-1]
    ranks = np.arange(e) - gstart[gkey[order]]
    ecore = core[order]
    epos = chunk_base[blk[order], half[order]] * P + ranks

    src_pad = np.zeros((cfg.NCORES, S_pad * P), np.int16)
    drel_pad = np.full((cfg.NCORES, S_pad * P), DEAD, np.float32)
    ea_pad = np.zeros((cfg.NCORES, S_pad * P, ED), np.float32)
    src_pad[ecore, epos] = (src[order] - half[order] * cfg.SPLIT).astype(np.int16)
    drel_pad[ecore, epos] = lane[order].astype(np.float32)
    ea_pad[ecore, epos] = np.asarray(edge_attr, np.float32)[order]

    # wrapped idx layout: flat slot k -> [k % 16, k // 16], replicated x8
    k = np.arange(CPT * P)
    wrapped = np.zeros((cfg.NCORES, T, 16, CPT * P // 16), np.int16)
    wrapped[:, :, k % 16, k // 16] = src_pad.reshape(cfg.NCORES, T, CPT * P)
    gidx = np.tile(wrapped, (1, 1, 8, 1))                 # [NC, T, 128, 64]

    drelT = drel_pad.reshape(cfg.NCORES, T, CPT, P).transpose(0, 1, 3, 2)
    drow = drel_pad.reshape(cfg.NCORES, T, CPT * P).astype(BF)
    eaT = ea_pad.reshape(cfg.NCORES, T, CPT * P, ED).transpose(0, 1, 3, 2)

    aux = np.zeros((cfg.NCORES, T, P, AUXB), np.uint8)
    aux[:, :, 0:BLK, 0:AUX_GIDX] = (
        drow[:, :, None, :].view(np.uint8).reshape(cfg.NCORES, T, 1, AUX_GIDX)
    )
    aux[:, :, BLK:P, 0:AUX_GIDX] = (
        np.ascontiguousarray(eaT.astype(BF)).view(np.uint8).reshape(
            cfg.NCORES, T, 16, AUX_GIDX)
    )
    aux[:, :, :, AUX_GIDX:AUX_DREL] = gidx.view(np.uint8).reshape(
        cfg.NCORES, T, P, 128)
    aux[:, :, :, AUX_DREL:AUXB] = (
        np.ascontiguousarray(drelT.astype(BF)).view(np.uint8).reshape(
            cfg.NCORES, T, P, CPT * 2))

    # gather runs per tile: maximal same-half chunk ranges
    runs = []
    for t in range(T):
        rr = []
        a = 0
        for c in range(1, CPT + 1):
            if c == CPT or chunk_half[t * CPT + c] != chunk_half[t * CPT + a]:
                rr.append((int(chunk_half[t * CPT + a]), a, c))
                a = c
        runs.append(rr)

    sched = dict(
        T=T,
        chunk_blk=chunk_blk.tolist(),
        first_chunk=first_chunk.tolist(),
        last_chunk=last_chunk.tolist(),
        runs=runs,
    )
    return sched, np.ascontiguousarray(aux)


def _nontriv(a, v):
    return not np.all(np.asarray(a) == v)


def build_trace(cfg, sched, weights, phases=("table", "xr", "edge", "node")):
    """Build the Bass/Tile program (identical for all cores)."""
    T = sched["T"]
    chunk_blk = sched["chunk_blk"]
    last_chunk_of = {g: b for b, g in enumerate(sched["last_chunk"])}
    first_chunk_of = {g: b for b, g in enumerate(sched["first_chunk"])}

    W = weights
    use_bl = _nontriv(W["bl"], 0.0)
    use_br = _nontriv(W["br"], 0.0)
    use_bgat = _nontriv(W["b_gat"], 0.0)
    use_g1 = _nontriv(W["g1"], 1.0)
    use_b1 = _nontriv(W["b1"], 0.0)
    use_bm1 = _nontriv(W["b_m1"], 0.0)
    use_gm = _nontriv(W["g_m"], 1.0)
    use_bm = _nontriv(W["b_m"], 0.0)
    use_bm2 = _nontriv(W["b_m2"], 0.0)
    use_g2 = _nontriv(W["g2"], 1.0)
    use_b2 = _nontriv(W["b2"], 0.0)

    nc = bacc.Bacc("TRN2", target_bir_lowering=False, debug=False,
                   num_swdge_queues=4)

    # ---------------- I/O declarations ----------------
    d_aux = nc.dram_tensor("aux", [T, P, AUXB], U8, kind="ExternalInput")
    d_xtg_hi = nc.dram_tensor("xtg_hi", [cfg.NG, P, G1 * P], BF16,
                              kind="ExternalInput")
    d_xtg_lo = nc.dram_tensor("xtg_lo", [cfg.NG, D - P, G1 * P], BF16,
                              kind="ExternalInput")
    d_xog_hi = nc.dram_tensor("xog_hi", [cfg.NBG, P, G2 * BLK], BF16,
                              kind="ExternalInput")
    d_xog_lo = nc.dram_tensor("xog_lo", [cfg.NBG, D - P, G2 * BLK], BF16,
                              kind="ExternalInput")
    d_xown = nc.dram_tensor("x_own", [cfg.NPAD, D], F32, kind="ExternalInput")
    d_out = nc.dram_tensor("y_out", [cfg.NPAD, D], F32, kind="ExternalOutput")

    def inline(arr, name):
        return nc.inline_tensor(np.ascontiguousarray(arr), name=name)

    bf = lambda a: np.asarray(a, np.float32).astype(BF)
    c_Wl_hi = inline(bf(W["Wl"][0:P, :]), "c_Wl_hi")
    c_Wl_lo = inline(bf(W["Wl"][P:D, :]), "c_Wl_lo")
    c_Wr_hi = inline(bf(W["Wr"][0:P, :]), "c_Wr_hi")
    c_Wr_lo = inline(bf(W["Wr"][P:D, :]), "c_Wr_lo")
    c_We = inline(bf(W["We"]), "c_We")
    c_ident = inline(np.eye(P, dtype=BF), "c_ident")
    c_att = inline(np.broadcast_to(
        bf(np.asarray(W["att"]).reshape(1, D)), (P, D)).copy(), "c_att")
    c_iota_c = inline(np.arange(BLK, dtype=np.float32).reshape(BLK, 1),
                      "c_iota_c")
    c_iota_r = inline(np.broadcast_to(
        np.arange(P, dtype=np.float32).reshape(1, P).astype(BF),
        (P, P)).copy(), "c_iota_r")
    c_Wm1_hi = inline(bf(W["W_m1"][0:P, :]), "c_Wm1_hi")
    c_Wm1_lo = inline(bf(W["W_m1"][P:D, :]), "c_Wm1_lo")
    c_Wm2 = inline(
        bf(W["W_m2"]).reshape(4, P, D).transpose(1, 0, 2).copy(), "c_Wm2"
    )  # [128, 4, 160]
    rows32 = lambda a: np.broadcast_to(
        np.asarray(a, np.float32).reshape(1, -1), (P, np.asarray(a).size)
    ).copy()
    c_bl = inline(rows32(W["bl"]), "c_bl")
    c_br = inline(rows32(W["br"]), "c_br")
    c_bgat = inline(rows32(W["b_gat"]), "c_bgat")
    c_g1 = inline(rows32(W["g1"]), "c_g1")
    c_b1 = inline(rows32(W["b1"]), "c_b1")
    c_bm1 = inline(rows32(W["b_m1"]), "c_bm1")
    c_gm = inline(rows32(W["g_m"]), "c_gm")
    c_bm = inline(rows32(W["b_m"]), "c_bm")
    c_bm2 = inline(rows32(W["b_m2"]), "c_bm2")
    c_g2 = inline(rows32(W["g2"]), "c_g2")
    c_b2 = inline(rows32(W["b2"]), "c_b2")

    with tile.TileContext(nc) as tc:
        psp = tc.alloc_tile_pool(name="psp", bufs=8, space="PSUM")
        dram = tc.alloc_tile_pool(name="dram", bufs=1, space="DRAM")
        xl_table = dram.tile([cfg.TROWS, EW], BF16, name="xl_table",
                             tag="xl_table")
        agg_d = dram.tile([cfg.NPAD, D], F32, name="agg_d", tag="agg_d")

        cp = tc.alloc_tile_pool(name="consts", bufs=1)

        def csb(dr, shape, dtype, name):
            t = cp.tile(shape, dtype, name=name, tag=name)
            nc.sync.dma_start(out=t[tuple(slice(0, s) for s in shape)],
                              in_=dr[:])
            return t

        Wl_hi = csb(c_Wl_hi, [P, D], BF16, "Wl_hi")
        Wl_lo = csb(c_Wl_lo, [D - P, D], BF16, "Wl_lo")
        Wr_hi = csb(c_Wr_hi, [P, D], BF16, "Wr_hi")
        Wr_lo = csb(c_Wr_lo, [D - P, D], BF16, "Wr_lo")
        ident = csb(c_ident, [P, P], BF16, "ident")
        att_sb = csb(c_att, [P, D], BF16, "att_sb")
        iota_c = csb(c_iota_c, [BLK, 1], F32, "iota_c")
        iota_r = csb(c_iota_r, [P, P], BF16, "iota_r")
        Wm1_hi = csb(c_Wm1_hi, [P, HID], BF16, "Wm1_hi")
        Wm1_lo = csb(c_Wm1_lo, [D - P, HID], BF16, "Wm1_lo")
        Wm2_sb = csb(c_Wm2, [P, 4, D], BF16, "Wm2_sb")
        bl_sb = csb(c_bl, [P, D], F32, "bl_sb") if use_bl else None
        br_sb = csb(c_br, [P, D], F32, "br_sb") if use_br else None
        bgat_sb = csb(c_bgat, [P, D], F32, "bgat_sb") if use_bgat else None
        g1_sb = csb(c_g1, [P, D], F32, "g1_sb") if use_g1 else None
        b1_sb = csb(c_b1, [P, D], F32, "b1_sb") if use_b1 else None
        bm1_sb = csb(c_bm1, [P, HID], F32, "bm1_sb") if use_bm1 else None
        gm_sb = csb(c_gm, [P, HID], F32, "gm_sb") if use_gm else None
        bm_sb = csb(c_bm, [P, HID], F32, "bm_sb") if use_bm else None
        bm2_sb = csb(c_bm2, [P, D], F32, "bm2_sb") if use_bm2 else None
        g2_sb = csb(c_g2, [P, D], F32, "g2_sb") if use_g2 else None
        b2_sb = csb(c_b2, [P, D], F32, "b2_sb") if use_b2 else None
        eps_sb = cp.tile([P, 1], F32, name="eps_sb", tag="eps_sb")
        nc.gpsimd.memset(eps_sb[:, :], float(EPS))
        lna_sb = cp.tile([P, 1], F32, name="lna_sb", tag="lna_sb")
        nc.gpsimd.memset(lna_sb[:, :], float(math.log(SELU_L * SELU_A)))

        # xr per block + We rows, concatenated along free dim
        rhs_all = cp.tile([P, cfg.NBLK * D], BF16, name="rhs_all",
                          tag="rhs_all")
        nc.sync.dma_start(
            out=rhs_all[BLK:P, :].rearrange("p (b f) -> p b f", f=D),
            in_=c_We[:].rearrange("p f -> p () f").to_broadcast(
                [ED, cfg.NBLK, D]))

        # ---------------- phase 1a: xl table ----------------
        tp = tc.alloc_tile_pool(name="tbl", bufs=3)
        if "table" in phases:
            for g in range(cfg.NG):
                xt_hi = tp.tile([P, G1 * P], BF16, name=f"xt_hi{g}",
                                tag="xt_hi", bufs=3)
                xt_lo = tp.tile([D - P, G1 * P], BF16, name=f"xt_lo{g}",
                                tag="xt_lo", bufs=3)
                nc.sync.dma_start(out=xt_hi[:, :], in_=d_xtg_hi[g])
                nc.scalar.dma_start(out=xt_lo[:, :], in_=d_xtg_lo[g])
                xlb = tp.tile([P, G1, EW], BF16, name=f"xlb{g}", tag="xlb",
                              bufs=3)
                for a in range(G1):
                    ps = psp.tile([P, D], F32, name=f"ps_xl{g}_{a}", tag="ps")
                    nc.tensor.matmul(ps[:, :], xt_hi[:, a * P:(a + 1) * P],
                                     Wl_hi[:, :], start=True, stop=False)
                    nc.tensor.matmul(ps[:, :], xt_lo[:, a * P:(a + 1) * P],
                                     Wl_lo[:, :], start=False, stop=True)
                    dst = xlb[:, a, 0:D]
                    if use_bl:
                        nc.vector.tensor_tensor(out=dst, in0=ps[:, :],
                                                in1=bl_sb[:, :], op=AL.add)
                    elif a % 2 == 0:
                        nc.scalar.copy(out=dst, in_=ps[:, :])
                    else:
                        nc.vector.tensor_copy(out=dst, in_=ps[:, :])
                nc.sync.dma_start(
                    out=xl_table[g * G1 * P:(g + 1) * G1 * P, :].rearrange(
                        "(a p) w -> p a w", p=P),
                    in_=xlb[:, :, :])

        # ---------------- phase 1b: xr per block -> rhs_all ----------------
        xp = tc.alloc_tile_pool(name="xrp", bufs=3)
        if "xr" in phases:
            for bg in range(cfg.NBG):
                nb = min(G2, cfg.NBLK - bg * G2)
                xo_hi = xp.tile([P, G2 * BLK], BF16, name=f"xo_hi{bg}",
                                tag="xo_hi", bufs=2)
                xo_lo = xp.tile([D - P, G2 * BLK], BF16, name=f"xo_lo{bg}",
                                tag="xo_lo", bufs=2)
                nc.scalar.dma_start(out=xo_hi[:, :], in_=d_xog_hi[bg])
                nc.scalar.dma_start(out=xo_lo[:, :], in_=d_xog_lo[bg])
                for bb in range(nb):
                    b = bg * G2 + bb
                    ps = psp.tile([BLK, D], F32, name=f"ps_xr{b}", tag="ps")
                    nc.tensor.matmul(ps[:, :],
                                     xo_hi[:, bb * BLK:(bb + 1) * BLK],
                                     Wr_hi[:, :], start=True, stop=False)
                    nc.tensor.matmul(ps[:, :],
                                     xo_lo[:, bb * BLK:(bb + 1) * BLK],
                                     Wr_lo[:, :], start=False, stop=True)
                    dst = rhs_all[0:BLK, b * D:(b + 1) * D]
                    if use_br:
                        nc.vector.tensor_tensor(out=dst, in0=ps[:, :],
                                                in1=br_sb[0:BLK, :], op=AL.add)
                    elif bb % 2 == 0:
                        nc.scalar.copy(out=dst, in_=ps[:, :])
                    else:
                        nc.vector.tensor_copy(out=dst, in_=ps[:, :])

        # ---------------- phase 3 emitter: node MLP group ----------------
        npo = tc.alloc_tile_pool(name="np", bufs=1)

        def emit_node_group(j0, j1):
                A = j1 - j0
                rs = slice(j0 * P, j1 * P)
                x_t = npo.tile([P, NA, D], F32, name=f"x{j0}", tag="x", bufs=2)
                a_t = npo.tile([P, NA, D], F32, name=f"a{j0}", tag="a", bufs=2)
                nc.sync.dma_start(
                    out=x_t[:, 0:A, :],
                    in_=d_xown[rs, :].rearrange("(a p) d -> p a d", p=P))
                nc.sync.dma_start(
                    out=a_t[:, 0:A, :],
                    in_=agg_d[rs, :].rearrange("(a p) d -> p a d", p=P))
                t0 = npo.tile([P, NA, D], F32, name=f"t0_{j0}", tag="t0",
                              bufs=2)
                nc.vector.tensor_tensor(out=t0[:, 0:A, :], in0=x_t[:, 0:A, :],
                                        in1=a_t[:, 0:A, :], op=AL.add)
                if use_bgat:
                    for a in range(A):
                        nc.gpsimd.tensor_tensor(
                            out=t0[:, a, :], in0=t0[:, a, :],
                            in1=bgat_sb[:, :], op=AL.add)

                def ln_stats(src, width, A, nm):
                    """Per-slot mean/rstd for src [P, A, width]."""
                    st = npo.tile([P, NA, 6], F32, name=f"st{nm}",
                                  tag=f"st{nm[0]}", bufs=2)
                    mv = npo.tile([P, NA, 2], F32, name=f"mv{nm}",
                                  tag=f"mv{nm[0]}", bufs=2)
                    for a in range(A):
                        nc.vector.bn_stats(out=st[:, a, :], in_=src[:, a, :])
                        nc.vector.bn_aggr(out=mv[:, a, :], in_=st[:, a, :])
                    sd = npo.tile([P, NA], F32, name=f"sd{nm}",
                                  tag=f"sd{nm[0]}", bufs=2)
                    nc.scalar.activation(out=sd[:, 0:A], in_=mv[:, 0:A, 1],
                                         func=AF.Sqrt, bias=eps_sb[:, 0:1])
                    rstd = npo.tile([P, NA], F32, name=f"rstd{nm}",
                                    tag=f"rstd{nm[0]}", bufs=2)
                    nc.vector.reciprocal(out=rstd[:, 0:A], in_=sd[:, 0:A])
                    return mv, rstd

                # ---- LN1 (sqrt table) ----
                mv1, rstd1 = ln_stats(t0, D, A, f"1_{j0}")
                out1 = npo.tile([P, NA, D], F32, name=f"o1_{j0}", tag="o1",
                                bufs=2)
                for a in range(A):
                    nc.vector.scalar_tensor_tensor(
                        out=out1[:, a, :], in0=t0[:, a, :],
                        scalar=mv1[:, a, 0:1],
                        in1=rstd1[:, a:a + 1].to_broadcast([P, D]),
                        op0=AL.subtract, op1=AL.mult)
                    if use_g1:
                        nc.vector.tensor_tensor(out=out1[:, a, :],
                                                in0=out1[:, a, :],
                                                in1=g1_sb[:, :], op=AL.mult)
                    if use_b1:
                        nc.gpsimd.tensor_tensor(out=out1[:, a, :],
                                                in0=out1[:, a, :],
                                                in1=b1_sb[:, :], op=AL.add)
                o1b = npo.tile([P, NA, D], BF16, name=f"o1b{j0}", tag="o1b",
                               bufs=2)
                nc.scalar.copy(out=o1b[:, 0:A, :], in_=out1[:, 0:A, :])

                # ---- transposes + mm1 + selu (exp table) ----
                e_g = npo.tile([P, NA, HID], BF16, name=f"e{j0}", tag="e",
                               bufs=2)
                r_g = npo.tile([P, NA, HID], BF16, name=f"r{j0}", tag="r",
                               bufs=2)
                for a in range(A):
                    pt0 = psp.tile([P, P], BF16, name=f"pt0_{j0}_{a}",
                                   tag="ps")
                    nc.tensor.transpose(out=pt0[:, :], in_=o1b[:, a, 0:P],
                                        identity=ident[:, :])
                    t0s = npo.tile([P, P], BF16, name=f"t0s{j0}_{a}",
                                   tag="t0s", bufs=3)
                    nc.scalar.copy(out=t0s[:, :], in_=pt0[:, :])
                    pt1 = psp.tile([D - P, P], BF16, name=f"pt1_{j0}_{a}",
                                   tag="ps")
                    nc.tensor.transpose(out=pt1[:, :], in_=o1b[:, a, P:D],
                                        identity=ident[:, :])
                    t1s = npo.tile([D - P, P], BF16, name=f"t1s{j0}_{a}",
                                   tag="t1s", bufs=3)
                    nc.vector.tensor_copy(out=t1s[:, :], in_=pt1[:, :])
                    ps_h = psp.tile([P, HID], F32, name=f"ps_h{j0}_{a}",
                                    tag="ps")
                    nc.tensor.matmul(ps_h[:, :], t0s[:, :], Wm1_hi[:, :],
                                     start=True, stop=False)
                    nc.tensor.matmul(ps_h[:, :], t1s[:, :], Wm1_lo[:, :],
                                     start=False, stop=True)
                    src = ps_h[:, :]
                    if use_bm1:
                        y_sb = npo.tile([P, HID], F32, name=f"ysb{j0}_{a}",
                                        tag="ysb", bufs=2)
                        nc.vector.tensor_tensor(out=y_sb[:, :], in0=src,
                                                in1=bm1_sb[:, :], op=AL.add)
                        src = y_sb[:, :]
                    nc.scalar.activation(out=e_g[:, a, :], in_=src,
                                         func=AF.Exp, bias=lna_sb[:, 0:1])
                    nc.scalar.activation(out=r_g[:, a, :], in_=src,
                                         func=AF.Relu, scale=float(SELU_L))
                u2 = npo.tile([P, NA, HID], BF16, name=f"u2_{j0}", tag="u2",
                              bufs=2)
                nc.vector.scalar_tensor_tensor(
                    out=u2[:, 0:A, :], in0=e_g[:, 0:A, :],
                    scalar=float(SELU_L * SELU_A),
                    in1=r_g[:, 0:A, :], op0=AL.min, op1=AL.add)

                # ---- LN2 (sqrt table + identity-scale) ----
                mv2, rstd2 = ln_stats(u2, HID, A, f"2_{j0}")
                nb2 = npo.tile([P, NA], F32, name=f"nb2_{j0}", tag="nb2",
                               bufs=2)
                nc.vector.scalar_tensor_tensor(
                    out=nb2[:, 0:A], in0=mv2[:, 0:A, 0], scalar=-1.0,
                    in1=rstd2[:, 0:A], op0=AL.mult, op1=AL.mult)
                h_bf = npo.tile([P, NA, HID], BF16, name=f"h{j0}", tag="h",
                                bufs=2)
                for a in range(A):
                    nc.scalar.activation(
                        out=h_bf[:, a, :], in_=u2[:, a, :], func=AF.Identity,
                        bias=nb2[:, a:a + 1], scale=rstd2[:, a:a + 1])
                    if use_gm:
                        nc.vector.tensor_tensor(out=h_bf[:, a, :],
                                                in0=h_bf[:, a, :],
                                                in1=gm_sb[:, :], op=AL.mult)
                    if use_bm:
                        nc.gpsimd.tensor_tensor(out=h_bf[:, a, :],
                                                in0=h_bf[:, a, :],
                                                in1=bm_sb[:, :], op=AL.add)

                # ---- transposes + mm2 + residual ----
                t2 = npo.tile([P, NA, D], F32, name=f"t2_{j0}", tag="t2",
                              bufs=2)
                for a in range(A):
                    ps_m = psp.tile([P, D], F32, name=f"ps_m{j0}_{a}",
                                    tag="ps")
                    for k in range(4):
                        pth = psp.tile([P, P], BF16, name=f"pth{j0}_{a}_{k}",
                                       tag="ps")
                        nc.tensor.transpose(out=pth[:, :],
                                            in_=h_bf[:, a, k * P:(k + 1) * P],
                                            identity=ident[:, :])
                        hts = npo.tile([P, P], BF16, name=f"hts{j0}_{a}_{k}",
                                       tag="hts", bufs=4)
                        if k % 2 == 0:
                            nc.scalar.copy(out=hts[:, :], in_=pth[:, :])
                        else:
                            nc.vector.tensor_copy(out=hts[:, :], in_=pth[:, :])
                        nc.tensor.matmul(ps_m[:, :], hts[:, :],
                                         Wm2_sb[:, k, :],
                                         start=(k == 0), stop=(k == 3))
                    nc.vector.tensor_tensor(out=t2[:, a, :],
                                            in0=out1[:, a, :],
                                            in1=ps_m[:, :], op=AL.add)
                    if use_bm2:
                        nc.gpsimd.tensor_tensor(out=t2[:, a, :],
                                                in0=t2[:, a, :],
                                                in1=bm2_sb[:, :], op=AL.add)

                # ---- LN3 (sqrt table) ----
                mv3, rstd3 = ln_stats(t2, D, A, f"3_{j0}")
                y_g = npo.tile([P, NA, D], F32, name=f"y{j0}", tag="y",
                               bufs=2)
                for a in range(A):
                    nc.vector.scalar_tensor_tensor(
                        out=y_g[:, a, :], in0=t2[:, a, :],
                        scalar=mv3[:, a, 0:1],
                        in1=rstd3[:, a:a + 1].to_broadcast([P, D]),
                        op0=AL.subtract, op1=AL.mult)
                    if use_g2:
                        nc.vector.tensor_tensor(out=y_g[:, a, :],
                                                in0=y_g[:, a, :],
                                                in1=g2_sb[:, :], op=AL.mult)
                    if use_b2:
                        nc.gpsimd.tensor_tensor(out=y_g[:, a, :],
                                                in0=y_g[:, a, :],
                                                in1=b2_sb[:, :], op=AL.add)
                nc.sync.dma_start(
                    out=d_out[rs, :].rearrange("(a p) d -> p a d", p=P),
                    in_=y_g[:, 0:A, :])

        # ---------------- phase 2: edges (node groups interleaved) --------
        # node group (j0, j1) becomes ready once the edge tile holding the
        # last chunk of its last source block has been emitted.
        groups = [(j0, min(j0 + NA, cfg.NTILE))
                  for j0 in range(0, cfg.NTILE, NA)]
        ready_after = {}
        for (j0, j1) in groups:
            bmax = (j1 * P - 1) // BLK
            t_req = sched["last_chunk"][bmax] // CPT
            ready_after.setdefault(t_req, []).append((j0, j1))

        ep = tc.alloc_tile_pool(name="ep", bufs=1)
        if "edge" in phases:
            agg_tiles = {}
            for t in range(T):
                aux = ep.tile([P, AUXB], U8, name=f"aux{t}", tag="aux", bufs=6)
                nc.sync.dma_start(out=aux[:, :], in_=d_aux[t])
                maskea = aux[:, 0:AUX_GIDX].bitcast(BF16)      # [128, 1024]
                gidx_v = aux[:, AUX_GIDX:AUX_DREL].bitcast(I16)  # [128, 64]
                drel_v = aux[:, AUX_DREL:AUXB].bitcast(BF16)     # [128, 8]

                # one-hot mask rows (in place over drow) + m2 one-hots
                nc.vector.tensor_scalar(
                    out=maskea[0:BLK, :], in0=maskea[0:BLK, :],
                    scalar1=iota_c[:, 0:1], scalar2=None, op0=AL.is_equal)
                m2 = ep.tile([P, CPT, P], BF16, name=f"m2_{t}", tag="m2",
                             bufs=3)
                nc.vector.tensor_tensor(
                    out=m2[:, :, :],
                    in0=iota_r[:, :].rearrange("p n -> p () n").to_broadcast(
                        [P, CPT, P]),
                    in1=drel_v[:, :].rearrange("p c -> p c ()").to_broadcast(
                        [P, CPT, P]),
                    op=AL.is_equal)

                xlg = ep.tile([P, CPT, EW], BF16, name=f"xlg{t}", tag="xlg",
                              bufs=6)
                for (hf, a, b) in sched["runs"][t]:
                    nidx = P * (b - a)
                    nc.gpsimd.dma_gather(
                        out_ap=xlg[:, a:b, :],
                        in_ap=(xl_table[0:cfg.SPLIT, :] if hf == 0
                               else xl_table[cfg.SPLIT:cfg.TROWS, :]),
                        idxs_ap=gidx_v[:, a * CPT:b * CPT],
                        num_idxs=nidx, num_idxs_reg=nidx, elem_size=EW,
                        queue_num=t % 4)

                f_sb = ep.tile([P, CPT, D], BF16, name=f"f{t}", tag="f",
                               bufs=3)
                for cp2 in range(CPT // 2):
                    ps_s = psp.tile([P, 2, D], F32, name=f"ps_s{t}_{cp2}",
                                    tag="ps")
                    for i in range(2):
                        c = cp2 * 2 + i
                        b = chunk_blk[t * CPT + c]
                        nc.tensor.matmul(ps_s[:, i, :],
                                         maskea[:, c * P:(c + 1) * P],
                                         rhs_all[:, b * D:(b + 1) * D],
                                         start=True, stop=False)
                        nc.tensor.matmul(ps_s[:, i, :], ident[:, :],
                                         xlg[:, c, 0:D],
                                         start=False, stop=True)
                    nc.scalar.activation(
                        out=f_sb[:, cp2 * 2:cp2 * 2 + 2, :],
                        in_=ps_s[:, :, :], func=AF.Lrelu, alpha=float(SLOPE))

                # fa = f * att (in place), per-head reduce -> scores
                nc.vector.tensor_tensor(
                    out=f_sb[:, :, :], in0=f_sb[:, :, :],
                    in1=att_sb[:, :].rearrange("p f -> p () f").to_broadcast(
                        [P, CPT, D]),
                    op=AL.mult)
                sc = ep.tile([P, CPT, H], BF16, name=f"sc{t}", tag="sc",
                             bufs=3)
                with nc.allow_low_precision(reason="32-way bf16 score reduce"):
                    nc.vector.tensor_reduce(
                        out=sc[:, :, :],
                        in_=f_sb[:, :, :].rearrange(
                            "p c (h z) -> p c h z", z=C),
                        axis=mybir.AxisListType.X, op=AL.add)

                w_sb = ep.tile([P, CPT, D + H], BF16, name=f"w{t}", tag="w",
                               bufs=3)
                nc.scalar.activation(
                    out=w_sb[:, :, 0:D].rearrange("p c (h z) -> p c h z", z=C),
                    in_=sc[:, :, :].rearrange("p c h -> p c h ()").to_broadcast(
                        [P, CPT, H, C]),
                    func=AF.Exp)
                nc.scalar.activation(out=w_sb[:, :, D:D + H], in_=sc[:, :, :],
                                     func=AF.Exp)
                nc.vector.tensor_tensor(
                    out=w_sb[:, :, 0:D], in0=w_sb[:, :, 0:D],
                    in1=xlg[:, :, 0:D], op=AL.mult)

                for c in range(CPT):
                    g = t * CPT + c
                    b = chunk_blk[g]
                    if g in first_chunk_of:
                        agg_tiles[b] = psp.tile([P, D + H], F32,
                                                name=f"agg{b}", tag="ps")
                    at = agg_tiles[b]
                    nc.tensor.matmul(at[:, :], m2[:, c, :], w_sb[:, c, :],
                                     start=(g in first_chunk_of),
                                     stop=(g in last_chunk_of))
                    if g in last_chunk_of:
                        den = ep.tile([BLK, H], F32, name=f"den{b}", tag="den",
                                      bufs=2)
                        nc.vector.tensor_scalar_add(
                            out=den[:, :], in0=at[0:BLK, D:D + H],
                            scalar1=1e-30)
                        rec = ep.tile([BLK, H], F32, name=f"rec{b}", tag="rec",
                                      bufs=2)
                        nc.vector.reciprocal(out=rec[:, :], in_=den[:, :])
                        aggn = ep.tile([BLK, D], F32, name=f"aggn{b}",
                                       tag="aggn", bufs=2)
                        nc.vector.tensor_tensor(
                            out=aggn[:, :].rearrange("p (h z) -> p h z", z=C),
                            in0=at[0:BLK, 0:D].rearrange(
                                "p (h z) -> p h z", z=C),
                            in1=rec[:, :].rearrange(
                                "p h -> p h ()").to_broadcast([BLK, H, C]),
                            op=AL.mult)
                        nc.sync.dma_start(
                            out=agg_d[b * BLK:(b + 1) * BLK, :],
                            in_=aggn[:, :])
                        del agg_tiles[b]

                if "node" in phases:
                    for (j0, j1) in ready_after.get(t, []):
                        emit_node_group(j0, j1)
        elif "node" in phases:
            for (j0, j1) in groups:
                emit_node_group(j0, j1)

        ep.release()
        npo.release()
        xp.release()
        tp.release()
        psp.release()
        dram.release()
        cp.release()

    nc.compile()
    return nc


def _make_in_maps(cfg, x, aux):
    x32 = np.asarray(x, np.float32)
    xbf = x32.astype(BF)
    xT = np.zeros((D, cfg.TROWS), BF)
    xT[:, :cfg.N] = xbf.T
    # fused tile groups: [NG, feat, G1, 128] -> [NG, feat, G1*128]
    xtg = xT.reshape(D, cfg.NG, G1 * P).transpose(1, 0, 2)
    in_maps = []
    for k in range(cfg.NCORES):
        xo = np.zeros((cfg.NPAD, D), np.float32)
        xo[:cfg.NV] = x32[k * cfg.NV:(k + 1) * cfg.NV]
        xoT = np.zeros((D, cfg.NBG * G2 * BLK), BF)
        xoT[:, :cfg.NV] = xbf[k * cfg.NV:(k + 1) * cfg.NV].T
        xog = xoT.reshape(D, cfg.NBG, G2 * BLK).transpose(1, 0, 2)
        in_maps.append({
            "aux": aux[k],
            "xtg_hi": np.ascontiguousarray(xtg[:, 0:P]),
            "xtg_lo": np.ascontiguousarray(xtg[:, P:D]),
            "xog_hi": np.ascontiguousarray(xog[:, 0:P]),
            "xog_lo": np.ascontiguousarray(xog[:, P:D]),
            "x_own": xo,
        })
    return in_maps


def build_all(inputs, cfg=None, phases=("table", "xr", "edge", "node")):
    cfg = cfg or Cfg()
    sched, aux = _prep_edges(cfg, inputs["edge_index"], inputs["edge_attr"])
    wnames = ["Wl", "bl", "Wr", "br", "We", "att", "b_gat", "g1", "b1",
              "W_m1", "b_m1", "g_m", "b_m", "W_m2", "b_m2", "g2", "b2"]
    weights = {k: np.asarray(inputs[k], np.float32) for k in wnames}
    nc = build_trace(cfg, sched, weights, phases=phases)
    in_maps = _make_in_maps(cfg, inputs["x"], aux)
    return cfg, nc, in_maps


def kernel(**inputs) -> np.ndarray:
    cfg, nc, in_maps = build_all(inputs)
    res = run_bass_kernel_spmd(nc, in_maps, core_ids=list(range(cfg.NCORES)))
    out = np.concatenate(
        [r["y_out"][:cfg.NV] for r in res.results], axis=0
    ).astype(np.float32)
    return out
